# revision 1
# baseline (speedup 1.0000x reference)
"""DSConv (dynamic snake conv) forward on 8 Trainium2 NeuronCores.

Self-contained: hardcodes all shapes from the problem spec.
  f[4,64,128,128] -> offset conv(9x9, 64->18) -> BN(batch stats, needs
  cross-core allreduce) -> tanh -> cumulative offsets -> two data-dependent
  gathers -> two 9-tap convs -> sum.

Sharding: core r = (b = r//2, w-half = r%2). Each core computes a
[64, 64, 128] output block. All index arithmetic is arranged so the compiled
program is identical on every core (per-core-ness lives in host-sliced
inputs): the gather table window for core (b, W0) starts at flat_aug row
b*16384 + W0*136, which makes every index base a compile-time constant.
"""
import sys
sys.path.insert(0, '/opt/trn_rl_repo')
import os
PHASE = int(os.environ.get("DSCONV_PHASE", "4"))

import numpy as np

import concourse.bass as bass
import concourse.tile as tile
from concourse import bacc, mybir
from concourse import bass_utils

# problem constants
K, PAD = 9, 4
B, C, OUT = 4, 64, 64
H = W = 128
WIDTH = H + 2 * PAD          # 136
NW = 64                      # output w-rows per core
FR = NW + K - 1              # 72 f-rows needed per core
FCOLS = FR * WIDTH           # 9792
NPOS = NW * H                # 8192 output positions per core
WIN = 10752                  # gather-table window rows (mult of 128)
NAUG = B * WIDTH * WIDTH + 548   # 74532 augmented flat rows
NE = 2 * K                   # 18 offset channels
NKY = 162                    # (ky, e) matmul column count
NCHUNK = 5                   # ceil(64c * 9kx / 128)
NBN = float(B * H * W)       # BN normalizer 65536

# compile-time index bases (see _prep_inputs for the window layout)
KB2 = [(8 - k) * 136 + k for k in range(K)]   # gather2 base per k (+137*fl)
KB3 = [k * 137 for k in range(K)]             # gather3 base per k (+136*ft+fu)
C2_STATIC = {k: KB2[k] + 4 * 137 for k in (0, 8)}
C3_STATIC = {k: KB3[k] + 4 * 136 + 4 for k in (0, 8)}

_NC = None  # cached compiled program


def _ap(x, off, dims):
    """Free-dim view of an SBUF tile AP (keeps its partition dim)."""
    return bass.AP(x.tensor, x.offset + off, [list(x.ap[0])] + [list(d) for d in dims])


def _build_program():
    nc = bacc.Bacc("TRN2", target_bir_lowering=False, debug=False, num_devices=8)
    f32, f16, i16, i32 = (mybir.dt.float32, mybir.dt.float16,
                          mybir.dt.int16, mybir.dt.int32)

    fslab_h = nc.dram_tensor("fslab", [64, FCOLS], f32, kind="ExternalInput")
    fslabs_h = nc.dram_tensor("fslabs", [64, FCOLS], f32, kind="ExternalInput")
    table_h = nc.dram_tensor("table", [WIN, 128], f16, kind="ExternalInput")
    w2_h = nc.dram_tensor("w2", [128, NCHUNK * NKY], f32, kind="ExternalInput")
    wxy_h = nc.dram_tensor("wxy", [128, 2 * K * 64], f16, kind="ExternalInput")
    kb_h = nc.dram_tensor("kb", [128, 14], f32, kind="ExternalInput")
    bnw_h = nc.dram_tensor("bnw", [128, 36], f32, kind="ExternalInput")
    bias_h = nc.dram_tensor("bias", [64, 1], f32, kind="ExternalInput")
    out_h = nc.dram_tensor("out", [64, NPOS], f32, kind="ExternalOutput")
    dbg_h = nc.dram_tensor("dbg", [128, 4 * 448], f32, kind="ExternalOutput") if PHASE < 4 else None

    with tile.TileContext(nc) as tc:
        _body(nc, tc, fslab_h, fslabs_h, table_h, w2_h, wxy_h, kb_h, bnw_h,
              bias_h, out_h, dbg_h)
    nc.compile()
    return nc


def _body(nc, tc, fslab_h, fslabs_h, table_h, w2_h, wxy_h, kb_h, bnw_h,
          bias_h, out_h, dbg_h=None):
    f32, f16, i16, i32 = (mybir.dt.float32, mybir.dt.float16,
                          mybir.dt.int16, mybir.dt.int32)
    from contextlib import ExitStack

    with ExitStack() as ctx:
        persist = ctx.enter_context(tc.tile_pool(name="persist", bufs=1))
        dram = ctx.enter_context(tc.tile_pool(name="dram", bufs=1, space="DRAM"))

        # persistent tiles
        slabT = persist.tile([128, WIN], f16)          # table transposed (ch-major)
        w2sb = persist.tile([128, NCHUNK * NKY], f32)
        wxysb = persist.tile([128, 2 * K * 64], f16)
        kbsb = persist.tile([128, 14], f32)
        bnwsb = persist.tile([128, 36], f32)
        biassb = persist.tile([64, 1], f32)
        ones = persist.tile([128, 128], f32)
        out_sb = persist.tile([64, NPOS], f32)
        off_n = persist.tile([128, NW * NE], f32)      # [h, (w', e)]
        czb = persist.tile([128, 7 * NW], f32)         # [h, (k-1, w')], k=1..7
        cwb = persist.tile([128, 7 * NW], f32)
        idx_t2 = persist.tile([128, 7 * 512], i16)     # wrapped+replicated idxs
        idx_t3 = persist.tile([128, 7 * 512], i16)
        scale_t = persist.tile([128, 18], f32)
        shift_t = persist.tile([128, 18], f32)

        nc.sync.dma_start(slabT[:], table_h.ap(), transpose=True)
        nc.sync.dma_start(w2sb[:], w2_h.ap())
        nc.sync.dma_start(wxysb[:], wxy_h.ap())
        nc.sync.dma_start(kbsb[:], kb_h.ap())
        nc.sync.dma_start(bnwsb[:], bnw_h.ap())
        nc.sync.dma_start(biassb[:], bias_h.ap())
        nc.gpsimd.memset(ones[:], 1.0)

        # ---------------- phase 1: offset conv ----------------
        with ExitStack() as c1:
            convp = c1.enter_context(tc.tile_pool(name="convp", bufs=1))
            psc = c1.enter_context(tc.tile_pool(name="psc", bufs=4, space="PSUM"))

            fdup = convp.tile([128, FCOLS], f32)
            nc.sync.dma_start(fdup[0:64, :], fslab_h.ap())
            nc.sync.dma_start(fdup[64:128, :], fslabs_h.ap())
            p_sb = convp.tile([128, FR * NKY], f32)

            for w in range(FR):
                ps = psc.tile([128, NKY], f32, name="psconv")
                for j in range(NCHUNK):
                    lhsT = _ap(fdup[:], w * WIDTH + 2 * j, [[1, 128]])
                    nc.tensor.matmul(ps[:], lhsT, w2sb[:, j * NKY:(j + 1) * NKY],
                                     start=(j == 0), stop=(j == NCHUNK - 1))
                nc.vector.tensor_copy(p_sb[:, w * NKY:(w + 1) * NKY], ps[:])

            # ky-sum: off[h,(w',e)] = sum_ky P[h,(w'+ky)*162 + ky*18 + e]
            nc.vector.tensor_copy(off_n[:], _ap(p_sb[:], 0, [[NKY, NW], [1, NE]]))
            for ky in range(1, K):
                nc.vector.tensor_add(
                    off_n[:], off_n[:],
                    _ap(p_sb[:], ky * (NKY + NE), [[NKY, NW], [1, NE]]))

        if PHASE == 1:
            nc.sync.dma_start(bass.AP(dbg_h, 0, [[448 * 4, 128], [1, 1152]]),
                              off_n[:])
            return
        # ---------------- phase 2: BN stats + allreduce ----------------
        with ExitStack() as c2:
            bnp = c2.enter_context(tc.tile_pool(name="bnp", bufs=1))
            psb = c2.enter_context(tc.tile_pool(name="psb", bufs=1, space="PSUM"))

            sq = bnp.tile([128, NW * NE], f32)
            nc.vector.tensor_mul(sq[:], off_n[:], off_n[:])
            red = bnp.tile([128, 36], f32)
            # reduce over w' (axis layout [part][e][w'], reduce innermost)
            nc.vector.tensor_reduce(
                red[:, 0:18], _ap(off_n[:], 0, [[1, NE], [NE, NW]]),
                axis=mybir.AxisListType.X, op=mybir.AluOpType.add)
            nc.vector.tensor_reduce(
                red[:, 18:36], _ap(sq[:], 0, [[1, NE], [NE, NW]]),
                axis=mybir.AxisListType.X, op=mybir.AluOpType.add)

            ps_st = psb.tile([128, 36], f32, name="psstats")
            nc.tensor.matmul(ps_st[:], ones[:], red[:], start=True, stop=True)
            s_sb = bnp.tile([128, 36], f32)
            nc.vector.tensor_copy(s_sb[:], ps_st[:])

            drin = dram.tile([1, 36], f32, name="drin")
            drout = dram.tile([1, 36], f32, name="drout", addr_space="Shared")
            nc.gpsimd.dma_start(drin[:], s_sb[0:1, :])
            nc.gpsimd.collective_compute(
                "AllReduce", mybir.AluOpType.add,
                ins=[drin[:].opt()], outs=[drout[:].opt()],
                replica_groups=[[0, 1, 2, 3, 4, 5, 6, 7]])
            s_all = bnp.tile([128, 36], f32)
            nc.gpsimd.dma_start(
                s_all[:], bass.AP(drout[:].tensor, drout[:].offset,
                                  [[0, 128], [1, 36]]))

            mean = bnp.tile([128, 18], f32)
            ex2 = bnp.tile([128, 18], f32)
            nc.vector.tensor_scalar_mul(mean[:], s_all[:, 0:18], 1.0 / NBN)
            nc.vector.tensor_scalar_mul(ex2[:], s_all[:, 18:36], 1.0 / NBN)
            var = bnp.tile([128, 18], f32)
            nc.vector.tensor_mul(var[:], mean[:], mean[:])
            nc.vector.tensor_sub(var[:], ex2[:], var[:])
            nc.vector.tensor_scalar_add(var[:], var[:], 1e-5)
            sqv = bnp.tile([128, 18], f32)
            nc.scalar.activation(sqv[:], var[:], mybir.ActivationFunctionType.Sqrt)
            rstd = bnp.tile([128, 18], f32)
            nc.vector.reciprocal(rstd[:], sqv[:])
            nc.vector.tensor_mul(scale_t[:], bnwsb[:, 0:18], rstd[:])
            nc.vector.tensor_mul(shift_t[:], mean[:], scale_t[:])
            nc.vector.tensor_sub(shift_t[:], bnwsb[:, 18:36], shift_t[:])

            # normalize + tanh (in place on off_n)
            nc.vector.tensor_mul(off_n[:], off_n[:],
                                 _ap(scale_t[:], 0, [[0, NW], [1, NE]]))
            nc.vector.tensor_add(off_n[:], off_n[:],
                                 _ap(shift_t[:], 0, [[0, NW], [1, NE]]))
            nc.scalar.activation(off_n[:], off_n[:],
                                 mybir.ActivationFunctionType.Tanh)

        if PHASE == 2:
            nc.sync.dma_start(bass.AP(dbg_h, 0, [[448 * 4, 128], [1, 1152]]),
                              off_n[:])
            return
        # ---------------- phase 3: cumsums + gather indices ----------------
        with ExitStack() as c3:
            ixp = c3.enter_context(tc.tile_pool(name="ixp", bufs=1))

            def zv(e):   # [128, 64] view of offset channel e (w'-major)
                return _ap(off_n[:], e, [[NE, NW]])

            def blk(t, k):
                return t[:, (k - 1) * NW:k * NW]

            for dst, e0 in ((czb, 0), (cwb, K)):
                nc.vector.tensor_copy(blk(dst, 1), zv(e0 + 1))
                nc.vector.tensor_add(blk(dst, 2), blk(dst, 1), zv(e0 + 2))
                nc.vector.tensor_add(blk(dst, 3), blk(dst, 2), zv(e0 + 3))
                nc.vector.tensor_copy(blk(dst, 7), zv(e0 + 7))
                nc.vector.tensor_add(blk(dst, 6), blk(dst, 7), zv(e0 + 6))
                nc.vector.tensor_add(blk(dst, 5), blk(dst, 6), zv(e0 + 5))
                nc.vector.tensor_add(blk(dst, 4), blk(dst, 3), blk(dst, 5))
                nc.vector.tensor_scalar_mul(blk(dst, 4), blk(dst, 4), 0.5)

            spi = ixp.tile([128, 7 * NW], i32)
            nc.gpsimd.iota(spi[:], pattern=[[0, 7], [WIDTH, NW]], base=0,
                           channel_multiplier=1)
            spf = ixp.tile([128, 7 * NW], f32)
            nc.vector.tensor_copy(spf[:], spi[:])
            spb2 = ixp.tile([128, 7 * NW], f32)
            spb3 = ixp.tile([128, 7 * NW], f32)
            nc.vector.tensor_add(spb2[:], spf[:],
                                 _ap(kbsb[:], 0, [[1, 7], [0, NW]]))
            nc.vector.tensor_add(spb3[:], spf[:],
                                 _ap(kbsb[:], 7, [[1, 7], [0, NW]]))

            af = ixp.tile([128, 7 * NW], f32)
            ai = ixp.tile([128, 7 * NW], i32)
            idxf = ixp.tile([128, 7 * NW], f32)
            idx16 = ixp.tile([128, 7 * NW], i16)
            afu = ixp.tile([128, 7 * NW], f32)
            aiu = ixp.tile([128, 7 * NW], i32)

            # branch 2: idx = spb2 + 137 * rne(cz + 3.5)
            nc.vector.tensor_scalar_add(af[:], czb[:], 3.5)
            nc.vector.tensor_copy(ai[:], af[:])
            nc.vector.tensor_copy(af[:], ai[:])
            nc.vector.scalar_tensor_tensor(idxf[:], af[:], 137.0, spb2[:],
                                           op0=mybir.AluOpType.mult,
                                           op1=mybir.AluOpType.add)
            nc.vector.tensor_copy(idx16[:], idxf[:])
            _fold_idx(nc, idx16, idx_t2)

            # branch 3: idx = spb3 + 136 * rne(cw + 3.5) + rne(3.5 - cw)
            nc.vector.tensor_scalar(afu[:], cwb[:], -1.0, 3.5,
                                    op0=mybir.AluOpType.mult,
                                    op1=mybir.AluOpType.add)
            nc.vector.tensor_copy(aiu[:], afu[:])
            nc.vector.tensor_copy(afu[:], aiu[:])
            nc.vector.tensor_add(afu[:], afu[:], spb3[:])
            nc.vector.tensor_scalar_add(af[:], cwb[:], 3.5)
            nc.vector.tensor_copy(ai[:], af[:])
            nc.vector.tensor_copy(af[:], ai[:])
            nc.vector.scalar_tensor_tensor(idxf[:], af[:], 136.0, afu[:],
                                           op0=mybir.AluOpType.mult,
                                           op1=mybir.AluOpType.add)
            nc.vector.tensor_copy(idx16[:], idxf[:])
            _fold_idx(nc, idx16, idx_t3)

        if PHASE == 3:
            dbg16 = bass.AP(dbg_h, 0, [[448 * 4, 128], [1, 224]]).bitcast(mybir.dt.int16)
            nc.sync.dma_start(dbg16, idx_t2[:, 0:448])
            dbg16b = bass.AP(dbg_h, 224, [[448 * 4, 128], [1, 224]]).bitcast(mybir.dt.int16)
            nc.sync.dma_start(dbg16b, idx_t3[:, 0:448])
            nc.sync.dma_start(bass.AP(dbg_h, 448, [[448 * 4, 128], [1, 448]]), czb[:])
            nc.sync.dma_start(bass.AP(dbg_h, 896, [[448 * 4, 128], [1, 448]]), cwb[:])
            return
        # ---------------- phase 4: gathers + final convs ----------------
        with ExitStack() as c4:
            gp = c4.enter_context(tc.tile_pool(name="gp", bufs=3))
            psf = c4.enter_context(tc.tile_pool(name="psf", bufs=1, space="PSUM"))

            table_ap = bass.AP(table_h, 0, [[128, WIN], [1, 128]])
            HHALF = NPOS // 2                      # 4096 cols per half

            for half in range(2):
                psum_big = psf.tile([64, HHALF], f32, name="psbig")
                passes = []
                for k in (0, 8):
                    passes.append((0, k, None))
                    passes.append((1, k, None))
                for k in range(1, 8):
                    passes.append((0, k, idx_t2))
                for k in range(1, 8):
                    passes.append((1, k, idx_t3))

                for pi, (br, k, idxt) in enumerate(passes):
                    first, last = pi == 0, pi == len(passes) - 1
                    lhsT = wxysb[:, (br * K + k) * 64:(br * K + k) * 64 + 64]
                    if idxt is None:
                        ck = C2_STATIC[k] if br == 0 else C3_STATIC[k]
                        for s in range(8):
                            h_hi = half * 4 + s // 2
                            woff = (s % 2) * 32 * WIDTH
                            rhs = _ap(slabT[:], ck + h_hi * 16 + woff,
                                      [[WIDTH, 32], [1, 16]])
                            nc.tensor.matmul(psum_big[:, s * 512:(s + 1) * 512],
                                             lhsT, rhs, start=first, stop=last)
                    else:
                        gb = gp.tile([128, HHALF], f16, name="gbuf")
                        nc.gpsimd.dma_gather(
                            out_ap=gb[:].unsqueeze(1),
                            in_ap=table_ap,
                            idxs_ap=idxt[:, (k - 1) * 512 + half * 256:
                                         (k - 1) * 512 + half * 256 + 256],
                            num_idxs=HHALF,
                            num_idxs_reg=HHALF,
                            elem_size=128,
                            transpose=True,
                            single_packet=False,
                        )
                        for s in range(8):
                            nc.tensor.matmul(psum_big[:, s * 512:(s + 1) * 512],
                                             lhsT, gb[:, s * 512:(s + 1) * 512],
                                             start=first, stop=last)

                # drain psum -> out_sb with bias, into [o, w'*128 + h] layout
                out_view = _ap(out_sb[:], half * 64,
                               [[16, 4], [128, NW], [1, 16]])
                nc.vector.tensor_scalar_add(out_view, psum_big[:], biassb[:])

            nc.sync.dma_start(out_h.ap(), out_sb[:])


def _fold_idx(nc, idx16, idx_t):
    """[128h, (k,w')] int16 -> wrapped [16, k*512 + h_hi*64 + w'] + replicas."""
    for r in range(8):
        src = idx16[16 * r:16 * (r + 1), :]
        dst = bass.AP(idx_t[:].tensor, idx_t[:].offset + r * 64,
                      [[list(idx_t[:].ap[0])[0], 16], [512, 7], [1, 64]])
        nc.sync.dma_start(dst, src)
    for m in range(1, 8):
        nc.sync.dma_start(idx_t[16 * m:16 * (m + 1), :], idx_t[0:16, :])


def _prep_inputs(inputs):
    f = np.ascontiguousarray(np.asarray(inputs["f"], np.float32))
    offset_w = np.asarray(inputs["offset_w"], np.float32)
    bn_gamma = np.asarray(inputs["bn_gamma"], np.float32)
    bn_beta = np.asarray(inputs["bn_beta"], np.float32)
    convx_w = np.asarray(inputs["convx_w"], np.float32)
    convx_b = np.asarray(inputs["convx_b"], np.float32)
    convy_w = np.asarray(inputs["convy_w"], np.float32)
    convy_b = np.asarray(inputs["convy_b"], np.float32)

    fp = np.pad(f, ((0, 0), (0, 0), (PAD, PAD), (PAD, PAD)))
    flat = fp.transpose(0, 2, 3, 1).reshape(-1, C)
    flat_aug = np.concatenate([flat[-548:], flat], axis=0)
    tab128 = np.zeros((NAUG, 128), np.float16)
    tab128[:, :C] = flat_aug.astype(np.float16)

    w2 = np.zeros((128, NCHUNK * NKY), np.float32)
    for j in range(NCHUNK):
        for g in range(2):
            kx = 2 * j + g
            if kx > 8:
                continue
            w2[g * 64:(g + 1) * 64, j * NKY:(j + 1) * NKY] = (
                offset_w[:, :, :, kx].transpose(1, 2, 0).reshape(C, NKY))

    wxy = np.zeros((128, 2 * K * 64), np.float16)
    for k in range(K):
        wxy[:C, (0 * K + k) * 64:(0 * K + k) * 64 + 64] = (
            convx_w[:, :, k, 0].T.astype(np.float16))
        wxy[:C, (1 * K + k) * 64:(1 * K + k) * 64 + 64] = (
            convy_w[:, :, 0, k].T.astype(np.float16))

    kb = np.tile(np.array([KB2[k] for k in range(1, 8)] +
                          [KB3[k] for k in range(1, 8)], np.float32), (128, 1))
    bnw = np.tile(np.concatenate([bn_gamma, bn_beta]).astype(np.float32), (128, 1))
    bias = (convx_b + convy_b).astype(np.float32).reshape(64, 1)

    in_maps = []
    for r in range(8):
        b, W0 = r // 2, (r % 2) * NW
        fsl = np.ascontiguousarray(fp[b, :, W0:W0 + FR, :].reshape(C, FCOLS))
        fss = np.zeros((C, FR, WIDTH), np.float32)
        fss[:, :, :-1] = fp[b, :, W0:W0 + FR, 1:]
        lo = b * (H * W) + W0 * WIDTH
        in_maps.append({
            "fslab": fsl,
            "fslabs": np.ascontiguousarray(fss.reshape(C, FCOLS)),
            "table": np.ascontiguousarray(tab128[lo:lo + WIN]),
            "w2": w2, "wxy": wxy, "kb": kb, "bnw": bnw, "bias": bias,
        })
    return in_maps


def get_program():
    global _NC
    if _NC is None:
        _NC = _build_program()
    return _NC


def run(inputs, trace=False):
    nc = get_program()
    in_maps = _prep_inputs(inputs)
    res = bass_utils.run_bass_kernel_spmd(
        nc, in_maps, core_ids=list(range(8)), trace=trace)
    out = np.empty((B, OUT, W, H), np.float32)
    for r in range(8):
        b, W0 = r // 2, (r % 2) * NW
        out[b, :, W0:W0 + NW, :] = res.results[r]["out"].reshape(OUT, NW, H)
    return out, res


def kernel(**inputs) -> np.ndarray:
    out, _ = run(inputs, trace=False)
    return out

